# revision 10
# baseline (speedup 1.0000x reference)
"""Butterfly depthwise-conv kernel for 8 Trainium2 NeuronCores.

Sharding: data-parallel over batch (B=8 -> 1 sample per core). Inside a core:
partitions = (H-half, channel): p<64 -> channel p rows [0, H/2); p>=64 ->
channel p-64 rows [H/2, H). Free axis = padded rows of one half:
row stride W+2, 2 left-pad cols, 1 halo row above/below the interior, one
zero guard row, then a 2-row "ones" region used by the bias matmul tap.

The state is kept in fp8e4 (e4m3) with static per-stage scales q_i; stage-i
weights are scaled by q_{i+1}/q_i so every eviction is a plain relu+add (the
final stage descales by 1/q6 exactly). Per stage the 9 depthwise taps run as
5 fp8 DoubleRow matmuls per (chunk, half): each DR matmul pairs two taps as
its two k-tiles (the rhs AP's tile dim strides between the taps' shifted
views); the 5th pass pairs the center tap with a bias tap whose rhs is a
constant-ones region and whose lhsT row 0 carries q_{i+1}*bias. lhsT m packs
BOTH branches (m 0:63 = branch a diag(w), 64:127 = branch b with the
butterfly permutation folded in), and the half-A/half-B matmuls run
concurrently on disjoint 64-row PE tiles. Eviction per half: relu (ACT for
half A, DVE fused-max for half B), a DMA SBUF->SBUF cross-partition copy to
align the branch-b half, and one add (DVE / Pool). Stage 5 descales, adds
the bf16 x residual, and streams fp32 to HBM.
"""

import json
import sys

sys.path.insert(0, "/opt/trn_rl_repo")

import ml_dtypes
import numpy as np

import concourse.bass as bass
import concourse.mybir as mybir
from concourse.ap import AP
from concourse.tile import TileContext
from concourse.bass_utils import run_bass_kernel_spmd

# ---------------------------------------------------------------------------
# Walrus in this container accepts at most ONE sem wait / update per
# instruction; Tile emits more. Rewrite the BIR JSON before serialization:
# hoist excess waits onto preceding same-engine NoOps and excess updates onto
# trailing same-engine NoOps (engine queues are FIFO; a NoOp's update fires
# after the preceding instruction completes).
_wsplit_counter = [0]


def _fresh_name():
    _wsplit_counter[0] += 1
    return f"I-wsplit-{_wsplit_counter[0]}"


def _nop(engine, debug, wait=None, update=None):
    return {
        "debug": debug,
        "engine": engine,
        "ins": [],
        "name": _fresh_name(),
        "opcode": "NoOp",
        "outs": [],
        "sync_info": {
            "on_update": [update] if update else [],
            "on_wait": [wait] if wait else [],
        },
    }


def _rewrite_bir(j):
    for fn in j["functions"]:
        for bb in fn["blocks"]:
            new_insts = []
            for inst in bb["instructions"]:
                si = inst.get("sync_info")
                pre, post = [], []
                if si:
                    waits = si.get("on_wait") or []
                    if len(waits) > 1:
                        for w in waits[:-1]:
                            pre.append(_nop(inst["engine"], inst.get("debug", 0), wait=w))
                        si["on_wait"] = [waits[-1]]
                    ups = si.get("on_update") or []
                    opc = inst.get("opcode", "")
                    if len(ups) > 1 and "DMA" not in opc and "Dma" not in opc:
                        for u in ups[1:]:
                            post.append(_nop(inst["engine"], inst.get("debug", 0), update=u))
                        si["on_update"] = ups[:1]
                new_insts.extend(pre)
                new_insts.append(inst)
                new_insts.extend(post)
            bb["instructions"] = new_insts
    return j


_orig_to_json_bytes = bass.Bass.to_json_bytes


def _patched_to_json_bytes(self, *a, **kw):
    raw = _orig_to_json_bytes(self, *a, **kw)
    return json.dumps(_rewrite_bir(json.loads(raw))).encode()


bass.Bass.to_json_bytes = _patched_to_json_bytes
# ---------------------------------------------------------------------------

C = 64
AF = mybir.ActivationFunctionType
ALU = mybir.AluOpType
DR = mybir.MatmulPerfMode.DoubleRow
E4 = ml_dtypes.float8_e4m3

# static per-stage state scales (state_i holds q_i * now_i in fp8e4m3) and the
# internal scale of the final stage's psum; calibrated so |q*now| < ~120 and
# all weight ratios q_{i+1}/q_i stay in e4m3's healthy normal range.
QS = [16.0, 32.0, 64.0, 128.0, 128.0, 128.0, 256.0]


def build_program(H, W, num_bf):
    """Emit the Bass program for one core (one batch sample)."""
    HALF = H // 2
    SW = W + 2  # padded row stride
    ROWS = HALF + 2  # interior + top/bottom halo rows
    L0 = (ROWS + 1) * SW  # + one zero guard row for the corner wrap read
    L = L0
    GROUP_ROWS = 4  # interior rows per evict group
    n_groups = HALF // GROUP_ROWS
    NCOL = GROUP_ROWS * W  # eviction columns per group (1024)
    # per (stage, pass) lhsT cols: 4 DR passes (2 tiles x 128) + 1 plain (128)
    WSTAGE = 4 * 256 + 128
    WCOLS = num_bf * WSTAGE

    nc = bass.Bass()
    xbf_ext = nc.declare_dram_parameter("xbf", [C, H, W], mybir.dt.bfloat16, isOutput=False)
    xpad_ext = nc.declare_dram_parameter("xpad", [128, L0], mybir.dt.float8e4, isOutput=False)
    wt_ext = nc.declare_dram_parameter("lhsT", [128, WCOLS], mybir.dt.float8e4, isOutput=False)
    bias_ext = nc.declare_dram_parameter("bias", [128, num_bf], mybir.dt.float32, isOutput=False)
    out_ext = nc.declare_dram_parameter("out", [C, H, W], mybir.dt.float32, isOutput=True)

    def interior(r):
        # free-axis element offset of interior row r (0-based), col 0
        return (r + 1) * SW + 2

    def rows_ap(tile, pslice, r0, nrows, base_off=0):
        """[pslice, nrows, W] view of interior rows r0..r0+nrows-1."""
        o = interior(r0) + base_off
        v = tile[pslice, o : o + nrows * SW]
        return v.rearrange("p (r w) -> p r w", w=SW)[:, :, 0:W]

    with TileContext(nc) as tc:
        with (
            tc.tile_pool(name="state", bufs=1) as state,
            tc.tile_pool(name="evict", bufs=3) as evict,
            tc.tile_pool(name="res", bufs=2) as res,
            tc.tile_pool(name="psA", bufs=2, space="PSUM") as psum_a,
            tc.tile_pool(name="psB", bufs=2, space="PSUM") as psum_b,
        ):
            now0 = state.tile([128, L], mybir.dt.float8e4)
            now1 = state.tile([128, L], mybir.dt.float8e4)
            wt = state.tile([128, WCOLS], mybir.dt.float8e4)
            bias_t = state.tile([128, num_bf], mybir.dt.float32)

            # zero only pads/halos of now1 (now0 arrives fully pre-padded)
            pads = now1[:, 0:L0].rearrange("p (r w) -> p r w", w=SW)
            nc.vector.memset(pads[:, :, 0:2], 0.0)
            nc.vector.memset(now1[:, 0:SW], 0.0)  # top halo row
            nc.vector.memset(now1[:, (HALF + 1) * SW : L0], 0.0)  # bottom halo + guard

            # weights: same data on partitions 0-63 and 64-127; split per
            # stage so stage-0 matmuls only wait for their own slice
            nc.gpsimd.dma_start(out=bias_t[:], in_=bias_ext[:])
            for st in range(num_bf):
                nc.gpsimd.dma_start(
                    out=wt[:, st * WSTAGE : (st + 1) * WSTAGE],
                    in_=wt_ext[:, st * WSTAGE : (st + 1) * WSTAGE],
                )

            # initial load: host-prepadded fp8 state, contiguous DMAs
            CHUNK_ROWS = 16
            nrows_total = ROWS + 1
            r = 0
            while r < nrows_total:
                r1 = min(r + (8 if r == 0 else CHUNK_ROWS), nrows_total)
                if nrows_total - r1 < 4:
                    r1 = nrows_total
                nc.sync.dma_start(
                    out=now0[:, r * SW : r1 * SW],
                    in_=xpad_ext[:, r * SW : r1 * SW],
                )
                r = r1

            def pass_geom(p, r0):
                """(tile0 base offset, tile-dim stride) for DR pass p at rows r0..r0+1."""
                if p == 0:
                    return interior(r0 - 1) - 1, 2  # (-1,-1) & (-1,+1)
                if p == 1:
                    return interior(r0) - 1, 2  # (0,-1) & (0,+1)
                if p == 2:
                    return interior(r0 + 1) - 1, 2  # (+1,-1) & (+1,+1)
                if p == 3:
                    return interior(r0 - 1), 2 * SW  # (-1,0) & (+1,0)
                return interior(r0), 0  # (0,0) as a plain (non-DR) matmul

            bufs = [now0, now1]
            for i in range(num_bf):
                src = bufs[i % 2]
                dst = bufs[(i + 1) % 2]
                last = i == num_bf - 1
                nten, noff = src[:, 0:1].tensor, src[:, 0:1].offset
                wten, woff = wt[:, 0:1].tensor, wt[:, 0:1].offset
                for g in range(n_groups):
                    ps_a = psum_a.tile([128, NCOL], mybir.dt.float32, tag="ps_a")
                    ps_b = psum_b.tile([128, NCOL], mybir.dt.float32)
                    for cp in range(2):
                        r0 = g * GROUP_ROWS + cp * 2
                        for p in range(5):
                            base, delta = pass_geom(p, r0)
                            wslice = woff + i * WSTAGE + p * 256
                            for ps, pb in ((ps_a, 0), (ps_b, 64)):
                                if p < 4:
                                    lhs = AP(wten, wslice + pb * WCOLS,
                                             [[WCOLS, 64], [128, 2], [1, 128]])
                                    rhs = AP(nten, noff + pb * L + base,
                                             [[L, 64], [delta, 2], [SW, 2], [1, W]])
                                    nc.tensor.matmul(
                                        ps[:, cp * 512 : (cp + 1) * 512],
                                        lhs, rhs,
                                        start=(p == 0), stop=False,
                                        perf_mode=DR,
                                    )
                                else:
                                    lhs = AP(wten, wslice + pb * WCOLS,
                                             [[WCOLS, 64], [1, 128]])
                                    rhs = AP(nten, noff + pb * L + base,
                                             [[L, 64], [SW, 2], [1, W]])
                                    nc.tensor.matmul(
                                        ps[:, cp * 512 : (cp + 1) * 512],
                                        lhs, rhs,
                                        start=False, stop=True,
                                    )
                    # ---- eviction of GROUP_ROWS rows, both halves ----
                    # relu_a on ACT, relu_b on DVE (bias folded into both via
                    # the per-partition bias operand); one DMA-pair builds
                    # w_ = (brB of half A | brA of half B) so each half's
                    # branch-add is partition-aligned; add_lo on DVE, add_hi
                    # on Pool.
                    r0 = g * GROUP_ROWS
                    bcol = bias_t[:, i : i + 1]
                    if not last:
                        edt = mybir.dt.float8e4
                        u_a = evict.tile([128, NCOL], edt, tag="u_a")
                        u_b = evict.tile([128, NCOL], edt, tag="u_b")
                        v = evict.tile([128, NCOL], edt, tag="v")
                        t = evict.tile([128, NCOL], edt, tag="t")
                        # relus alternate engines by group parity to balance
                        if g % 2 == 0:
                            nc.scalar.activation(u_a[:, :], ps_a[:, 0:NCOL], AF.Relu,
                                                 bias=bcol, scale=1.0)
                            nc.vector.tensor_scalar(u_b[:, :], ps_b[:, 0:NCOL],
                                                    bcol, 0.0, ALU.add, ALU.max)
                        else:
                            nc.vector.tensor_scalar(u_a[:, :], ps_a[:, 0:NCOL],
                                                    bcol, 0.0, ALU.add, ALU.max)
                            nc.scalar.activation(u_b[:, :], ps_b[:, 0:NCOL], AF.Relu,
                                                 bias=bcol, scale=1.0)
                        nc.sync.dma_start(out=v[0:64, :], in_=u_a[0:64, :])
                        nc.sync.dma_start(out=v[64:128, :], in_=u_b[0:64, :])
                        nc.sync.dma_start(out=t[0:64, :], in_=u_a[64:128, :])
                        nc.sync.dma_start(out=t[64:128, :], in_=u_b[64:128, :])
                        nc.vector.tensor_add(
                            rows_ap(dst, slice(0, 128), r0, GROUP_ROWS),
                            v[:, :].rearrange("p (r w) -> p r w", w=W),
                            t[:, :].rearrange("p (r w) -> p r w", w=W),
                        )
                        if g == 0:
                            # half-B row 0 -> half-A bottom halo
                            nc.vector.tensor_add(
                                dst[0:64, interior(HALF) : interior(HALF) + W],
                                v[64:128, 0:W],
                                t[64:128, 0:W],
                            )
                        if g == n_groups - 1:
                            # half-A last row -> half-B top halo
                            lo = (GROUP_ROWS - 1) * W
                            nc.vector.tensor_add(
                                dst[64:128, interior(-1) : interior(-1) + W],
                                v[0:64, lo : lo + W],
                                t[0:64, lo : lo + W],
                            )
                    else:
                        # final stage: descale by 1/q6, add branches and the
                        # bf16 x residual, store fp32.
                        inv = 1.0 / QS[num_bf]
                        edt = mybir.dt.bfloat16
                        u_a = evict.tile([128, NCOL], edt, tag="u_a")
                        u_b = evict.tile([128, NCOL], edt, tag="u_b")
                        v = evict.tile([128, NCOL], edt, tag="v")
                        t = evict.tile([128, NCOL], edt, tag="t")
                        xr = res.tile([128, NCOL], edt, tag="xr", bufs=4)
                        s = res.tile([128, NCOL], edt, tag="s")
                        og = res.tile([128, NCOL], mybir.dt.float32, tag="og")
                        nc.scalar.dma_start(
                            out=xr[0:64, :].rearrange("p (r w) -> p r w", w=W),
                            in_=xbf_ext[:, r0 : r0 + GROUP_ROWS, :],
                        )
                        nc.scalar.dma_start(
                            out=xr[64:128, :].rearrange("p (r w) -> p r w", w=W),
                            in_=xbf_ext[:, HALF + r0 : HALF + r0 + GROUP_ROWS, :],
                        )
                        nc.scalar.activation(u_a[:, :], ps_a[:, 0:NCOL], AF.Relu,
                                             bias=bcol, scale=inv)
                        nc.scalar.activation(u_b[:, :], ps_b[:, 0:NCOL], AF.Relu,
                                             bias=bcol, scale=inv)
                        nc.sync.dma_start(out=v[0:64, :], in_=u_a[0:64, :])
                        nc.sync.dma_start(out=v[64:128, :], in_=u_b[0:64, :])
                        nc.sync.dma_start(out=t[0:64, :], in_=u_a[64:128, :])
                        nc.sync.dma_start(out=t[64:128, :], in_=u_b[64:128, :])
                        nc.vector.tensor_add(s[:, :], v[:, :], t[:, :])
                        nc.vector.tensor_add(og[:, :], s[:, :], xr[:, :])
                        nc.sync.dma_start(
                            out=out_ext[:, r0 : r0 + GROUP_ROWS, :],
                            in_=og[0:64, :].rearrange("p (r w) -> p r w", w=W),
                        )
                        nc.sync.dma_start(
                            out=out_ext[:, HALF + r0 : HALF + r0 + GROUP_ROWS, :],
                            in_=og[64:128, :].rearrange("p (r w) -> p r w", w=W),
                        )
    return nc


def host_prep(weights, biases, masks, num_bf):
    """Fold butterfly masks and stage scales into per-pass lhsT matrices.

    Layout: [128 partitions, num_bf*(4*256+128)] fp8e4m3; partitions 64:128
    repeat 0:64 (one copy per PE row-tile). Per stage: 4 DR passes of
    [k-tile0 128 cols | k-tile1 128 cols], then the plain center-tap pass
    (128 cols). Returns (lhsT, bias) with bias[:, i] = q_{i+1}*b for ACT/DVE
    (the last stage's column holds the unscaled b for the 1/q6 descale)."""
    PASS_TAPS = [
        ((0, 0), (0, 2)),  # (-1,-1) & (-1,+1)
        ((1, 0), (1, 2)),  # ( 0,-1) & ( 0,+1)
        ((2, 0), (2, 2)),  # (+1,-1) & (+1,+1)
        ((0, 1), (2, 1)),  # (-1, 0) & (+1, 0)
        ((1, 1), None),  # ( 0, 0) plain
    ]
    slabs = []
    for i in range(num_bf):
        r = QS[i + 1] / QS[i]
        m = masks[i]
        for p, (t0, t1) in enumerate(PASS_TAPS):
            for tap in (t0, t1):
                if tap is None:
                    continue
                dy, dx = tap
                w1 = np.zeros((64, 128), dtype=np.float32)
                for c in range(C):
                    w1[c, c] = weights[i, 0, c, 0, dy, dx] * r
                    w1[m[c], 64 + c] = weights[i, 1, c, 0, dy, dx] * r
                slabs.append(w1)
    lhsT = np.concatenate(slabs, axis=1)
    lhsT = np.clip(lhsT, -240.0, 240.0)
    full = np.concatenate([lhsT, lhsT], axis=0).astype(E4)
    bias = np.zeros((128, num_bf), dtype=np.float32)
    for i in range(num_bf):
        q = QS[i + 1] if i < num_bf - 1 else 1.0
        bias[0:64, i] = biases[i, 0] * q
        bias[64:128, i] = biases[i, 1] * q
    return np.ascontiguousarray(full), np.ascontiguousarray(bias)


def _run(x_full, weights, biases, masks, H, W, num_bf, trace=False):
    nc = build_program(H, W, num_bf)
    lhsT, bias = host_prep(
        np.asarray(weights, dtype=np.float32),
        np.asarray(biases, dtype=np.float32),
        np.asarray(masks),
        num_bf,
    )
    n = x_full.shape[0]
    xbf = np.ascontiguousarray(x_full.astype(ml_dtypes.bfloat16))
    # pre-padded SBUF-layout fp8 state: [128 partitions, (HALF+3)*(W+2)]
    HALF, SW = H // 2, W + 2
    xq = np.clip(x_full * QS[0], -240.0, 240.0).astype(E4)
    xpad = np.zeros((n, 128, HALF + 3, SW), dtype=E4)
    xpad[:, 0:64, 1 : HALF + 1, 2 : 2 + W] = xq[:, :, 0:HALF, :]
    xpad[:, 64:128, 1 : HALF + 1, 2 : 2 + W] = xq[:, :, HALF:H, :]
    xpad[:, 0:64, HALF + 1, 2 : 2 + W] = xq[:, :, HALF, :]  # A bottom halo
    xpad[:, 64:128, 0, 2 : 2 + W] = xq[:, :, HALF - 1, :]  # B top halo
    xpad = np.ascontiguousarray(xpad.reshape(n, 128, (HALF + 3) * SW))
    in_maps = [
        {"xbf": xbf[b], "xpad": xpad[b], "lhsT": lhsT, "bias": bias}
        for b in range(n)
    ]
    r = run_bass_kernel_spmd(nc, in_maps, core_ids=list(range(n)), trace=trace)
    out = np.stack([r.results[b]["out"] for b in range(n)], axis=0)
    return out, r


def kernel(x, weights, biases, masks):
    x = np.asarray(x, dtype=np.float32)
    out, _ = _run(x, weights, biases, masks, H=256, W=256, num_bf=6)
    return out
